# revision 29
# baseline (speedup 1.0000x reference)
"""3-layer GCN (GCNConv x3 + linear head) on 8 Trainium2 NeuronCores.

v3 strategy (continuous-descgen pull design):
  - Nodes bin-packed into 392 blocks of <=128 (balanced by in-edge count),
    49 blocks/core, grouped into 7 superblocks (=source "pieces") of 7
    blocks. Features bf16, PRESCALED hfull[n] = dis[n]*H[n]; leaky-relu
    positive homogeneity folds dis factors into one Prelu epilogue per
    block (as v2).
  - Halo exchange is PIECEWISE: 7 AllGathers per layer boundary, one per
    superblock, each into its own Shared DRAM tensor hfp[L][p]
    ([8*896, 128] piece-major), issued as soon as that superblock's
    outputs land. Gather indices are piece-relative (< 7168, int16).
  - Layers 1-2 gather source rows with SWDGE dma_gather in PREPARE_ONLY
    mode: descriptor generation (the HW bottleneck, ~2.2-3 ns/desc
    aggregate over 4 queues) streams continuously on the GpSimd engine
    from t~0, while trigger_dma fires each source-piece's descriptors
    once its AllGather piece lands. Ring: dynamic_dma_scratch 32KiB ->
    2048 desc/queue; calls sized ~13 groups (1664 descs) so one round
    fits the ring untriggered.
  - Compute for layers 1-2 runs in 7 source-piece passes: per (block,
    piece) chunk matmuls accumulate in PSUM, then DVE adds into a
    per-block f32 SBUF accumulator aggT [d, c]. Pass 0 adds the
    self-loop (identity rhs); the last pass interleaves the finalize
    (z = aggW + bias-outer + diag*h_in, Prelu) so outputs stream out
    block-by-block and the next AllGather pieces pipeline.
  - One-hot chunk matrices are NOT stored: the DVE generates them per
    piece with a broadcast is_equal against a per-chunk target-column
    table (iota row vs colloc), output fp8. Saves ~12 MB/layer of HBM
    streaming and ~80 KB/partition of SBUF.
  - Layer 0 needs no gather: host pregathers dis[src]*x[src] in chunk
    order; block-major compute over superblocks, bounce writes feed the
    first piecewise AllGather.
"""

import numpy as np

N = 50000
E = 600000
D = 128
NCORES = 8
BPC = 49                      # blocks per core
NBLK = NCORES * BPC           # 392
PC_SLOTS = BPC * 128          # 6272
SB = 7                        # blocks per superblock
NSB = BPC // SB               # 7 superblocks (= source pieces)
PIECE_ROWS = NCORES * SB * 128  # 7168 rows per hfp piece
NEG_SLOPE = 0.2
NQ = 4                        # SWDGE queues
SCRATCH = 57344               # dma scratch bytes/partition -> 3584 desc/queue

_CACHE = {}
LAST_EXEC_NS = None
LAST_RESULTS = None


def _pack_graph(edge_index, x):
    import heapq
    import ml_dtypes

    bf16 = ml_dtypes.bfloat16

    row = np.ascontiguousarray(edge_index[0]).astype(np.int64)
    col = np.ascontiguousarray(edge_index[1]).astype(np.int64)
    deg_t = np.bincount(col, minlength=N).astype(np.int64)
    dis = (1.0 / np.sqrt(deg_t + 1.0)).astype(np.float64)

    # --- node -> (block, pos): greedy balanced bin packing by in-degree ---
    order = np.argsort(-deg_t, kind="stable")
    heap = [(0, b) for b in range(NBLK)]
    heapq.heapify(heap)
    nodecnt = np.zeros(NBLK, np.int64)
    load = np.zeros(NBLK, np.int64)
    blk_of = np.empty(N, np.int64)
    pos_of = np.empty(N, np.int64)
    for n in order:
        while True:
            _, b = heapq.heappop(heap)
            if nodecnt[b] < 128:
                break
        blk_of[n] = b
        pos_of[n] = nodecnt[b]
        nodecnt[b] += 1
        load[b] += deg_t[n]
        heapq.heappush(heap, (load[b], b))

    c_of = blk_of // BPC
    bl49_of = blk_of % BPC
    sbn_of = bl49_of // SB
    bl7_of = bl49_of % SB
    # gather offset within the node's piece tensor hfp[sbn]
    poff_of = c_of * (SB * 128) + bl7_of * 128 + pos_of
    # v2-style output slot (block-major per core), for unshard
    slot_of = blk_of * 128 + pos_of

    # --- edge grouping ---
    tb = blk_of[col]                      # target block 0..391
    sp = sbn_of[row]                      # source piece 0..6
    key = tb * NSB + sp
    eorder = np.argsort(key, kind="stable")
    key_s = key[eorder]
    kstart = np.searchsorted(key_s, np.arange(NBLK * NSB + 1))

    e_cnt = (kstart[1:] - kstart[:-1]).reshape(NBLK, NSB)   # [tb, sp]
    tot_cnt = e_cnt.sum(axis=1)                              # [tb]

    # chunks per (bl49, sp): max over cores so the program is SPMD-shared
    cpb12 = np.zeros((BPC, NSB), np.int64)
    for bl49 in range(BPC):
        for s in range(NSB):
            cnts = e_cnt[bl49::BPC, s] if False else e_cnt[np.arange(NCORES) * BPC + bl49, s]
            cpb12[bl49, s] = int(np.ceil(cnts.max() / 128)) if cnts.max() else 0
    # L0 chunks per bl49 (max over cores of total)
    cpb0 = np.zeros(BPC, np.int64)
    for bl49 in range(BPC):
        cpb0[bl49] = int(np.ceil(tot_cnt[np.arange(NCORES) * BPC + bl49].max() / 128))

    G12 = cpb12.sum(axis=0)               # groups per sp piece
    G12MAX = int(G12.max())
    NI12 = int(G12.sum()) * 128           # gather idxs per core (padded)
    G0 = int(cpb0.sum())                  # L0 groups total
    G0_sb = [int(cpb0[s * SB:(s + 1) * SB].sum()) for s in range(NSB)]
    G0SBMAX = max(G0_sb)

    # group offsets
    off12 = np.zeros((BPC, NSB), np.int64)   # group offset within piece sp
    for s in range(NSB):
        off12[:, s] = np.cumsum(cpb12[:, s]) - cpb12[:, s]
    spbase = np.cumsum(G12) - G12            # piece group base (global chunks)
    off0 = np.cumsum(cpb0) - cpb0            # L0 group offset

    # --- per-core tensors ---
    import ml_dtypes as _mld
    fp8 = _mld.float8_e4m3fn
    TG12 = int(G12.sum())
    idx12 = np.zeros((NCORES, NI12), np.int16)
    oh12 = np.zeros((NCORES, 128, TG12 * 128), fp8)
    oh0 = np.zeros((NCORES, 128, G0 * 128), fp8)
    xs = np.asarray(x, np.float64) * dis[:, None]
    xs_bf = xs.astype(bf16)
    xg0 = np.zeros((NCORES, 128, G0, D), bf16)

    for b in range(NBLK):
        cc = b // BPC
        bl49 = b % BPC
        # layer 1-2: per source piece
        for s in range(NSB):
            sub = eorder[kstart[b * NSB + s]:kstart[b * NSB + s + 1]]
            if len(sub) == 0:
                continue
            i = np.arange(len(sub))
            t = i // 128
            p = i % 128
            g = off12[bl49, s] + t
            idx12[cc, (int(spbase[s]) + g) * 128 + p] = poff_of[row[sub]].astype(np.int16)
            oh12[cc][p, (int(spbase[s]) + g) * 128 + pos_of[col[sub]]] = np.float32(1.0)
        # layer 0: all edges of block, pregathered stream
        sub = eorder[kstart[b * NSB]:kstart[(b + 1) * NSB]]
        if len(sub):
            i = np.arange(len(sub))
            t = i // 128
            p = i % 128
            g0 = off0[bl49] + t
            xg0[cc, p, g0, :] = xs_bf[row[sub]]
            oh0[cc][p, g0 * 128 + pos_of[col[sub]]] = np.float32(1.0)

    # wrap idx: [NCORES, NI] -> [NCORES, 128, NI/16]
    def wrap(a):
        ncc, ni = a.shape
        w = a.reshape(ncc, ni // 16, 16).transpose(0, 2, 1)        # [c,16,ni/16]
        w = np.tile(w, (1, 8, 1))                                   # [c,128,ni/16]
        return np.ascontiguousarray(w)

    # per-block column tensors (v2 verbatim)
    dis_slots = np.ones(NBLK * 128, np.float64)
    dis_slots[slot_of] = dis
    invdis2_slots = np.ones(NBLK * 128, np.float64)
    invdis2_slots[slot_of] = deg_t + 1.0
    xs_slots = np.zeros((NBLK * 128, D), np.float64)
    xs_slots[slot_of] = xs
    xsl = xs_slots.astype(bf16).reshape(NCORES, BPC, 128, D).transpose(0, 2, 1, 3)
    xsl = np.ascontiguousarray(xsl.reshape(NCORES, 128, BPC * D))

    dis_b = dis_slots.reshape(NCORES, BPC, 128).transpose(0, 2, 1)
    scl2 = np.ascontiguousarray((dis_b * dis_b).astype(np.float32))
    scl1 = np.ascontiguousarray(dis_b.astype(np.float32))
    diag = np.zeros((NCORES, 128, BPC * 128), bf16)
    invd = np.zeros((NCORES, 1, BPC * 128), bf16)
    iv2 = invdis2_slots.reshape(NCORES, BPC, 128)
    for cc in range(NCORES):
        for j in range(BPC):
            dg = iv2[cc, j]
            diag[cc, np.arange(128), j * 128 + np.arange(128)] = dg.astype(bf16)
            invd[cc, 0, j * 128:(j + 1) * 128] = np.sqrt(dg).astype(bf16)

    return dict(
        slot_of=slot_of,
        cpb12=cpb12, cpb0=cpb0, G12=G12, G12MAX=G12MAX, NI12=NI12,
        G0=G0, G0_sb=G0_sb, G0SBMAX=G0SBMAX, off12=off12, off0=off0,
        spbase=spbase, TG12=TG12,
        idx12=wrap(idx12), oh12=oh12, oh0=oh0,
        xg0=xg0.reshape(NCORES, 128, G0 * D),
        xsl=xsl, scl2=scl2, scl1=scl1, diag=diag, invd=invd,
    )


def _build_program(pk):
    import concourse.bacc as bacc
    import concourse.tile as tile
    import concourse.mybir as mybir

    f32 = mybir.dt.float32
    bf16 = mybir.dt.bfloat16
    fp8 = mybir.dt.float8e4
    i16 = mybir.dt.int16
    ALU = mybir.AluOpType
    AF = mybir.ActivationFunctionType

    cpb12 = pk["cpb12"]; cpb0 = pk["cpb0"]
    G12 = pk["G12"]; G12MAX = pk["G12MAX"]; NI12 = pk["NI12"]
    G0 = pk["G0"]; G0_sb = pk["G0_sb"]; G0SBMAX = pk["G0SBMAX"]
    off12 = pk["off12"]; off0 = pk["off0"]; spbase = pk["spbase"]
    TG12 = pk["TG12"]

    GTILE = G12MAX

    import os as _os
    _nlayers = int(_os.environ.get("GNN3_LAYERS", "3"))

    nc = bacc.Bacc("TRN2", target_bir_lowering=False, debug=False,
                   enable_asserts=True, num_devices=NCORES,
                   num_swdge_queues=NQ, dynamic_dma_scratch_size=SCRATCH)

    idx_d = nc.dram_tensor("idx12", [128, NI12 // 16], i16, kind="ExternalInput").ap()
    oh12_d = nc.dram_tensor("oh12", [128, TG12 * 128], fp8, kind="ExternalInput").ap()
    oh0_d = nc.dram_tensor("oh0", [128, G0 * 128], fp8, kind="ExternalInput").ap()
    xg0_d = nc.dram_tensor("xg0", [128, G0 * D], bf16, kind="ExternalInput").ap()
    xsl_d = nc.dram_tensor("xsl", [128, BPC * D], bf16, kind="ExternalInput").ap()
    diag_d = nc.dram_tensor("diag", [128, BPC * 128], bf16, kind="ExternalInput").ap()
    invd_d = nc.dram_tensor("invd", [1, BPC * 128], bf16, kind="ExternalInput").ap()
    scl2_d = nc.dram_tensor("scl2", [128, BPC], f32, kind="ExternalInput").ap()
    scl1_d = nc.dram_tensor("scl1", [128, BPC], f32, kind="ExternalInput").ap()
    w_d = [nc.dram_tensor(f"w{i}", [D, D], bf16, kind="ExternalInput").ap() for i in (1, 2, 3)]
    brow_d = [nc.dram_tensor(f"brow{i}", [1, D], bf16, kind="ExternalInput").ap() for i in (1, 2, 3)]
    iden_d = nc.dram_tensor("iden", [128, 128], fp8, kind="ExternalInput").ap()
    lwb_d = nc.dram_tensor("lwb", [128, D], f32, kind="ExternalInput").ap()
    out_d = nc.dram_tensor("out", [PC_SLOTS], f32, kind="ExternalOutput").ap()

    # per (sp): two rounds of one sub-gather call per queue (~1664
    # descs/call; one round fits the ring untriggered).
    NCALLS = 2 * NQ
    call_plan = []  # [sp] -> list of rounds of (q, ga, gb)
    for s in range(NSB):
        g = int(G12[s])
        spans = []
        base = 0
        for ci in range(NCALLS):
            take = (g - base + (NCALLS - ci) - 1) // (NCALLS - ci)
            spans.append((base, base + take))
            base += take
        rounds = []
        for r in range(2):
            rounds.append([(q, spans[r * NQ + q][0], spans[r * NQ + q][1])
                           for q in range(NQ)
                           if spans[r * NQ + q][1] > spans[r * NQ + q][0]])
        call_plan.append(rounds)

    with tile.TileContext(nc) as tc:
        with (
            tc.tile_pool(name="const", bufs=1) as cpool,
            tc.tile_pool(name="gpool", bufs=2) as gpool,      # L1/L2 gather tiles
            tc.tile_pool(name="g0pool", bufs=2) as g0pool,    # L0 per-block streams
            tc.tile_pool(name="ohp", bufs=2) as ohp,          # L1/L2 one-hots
            tc.tile_pool(name="oh0p", bufs=2) as oh0p,        # L0 per-block one-hots
            tc.tile_pool(name="ep", bufs=4) as ep,
            tc.tile_pool(name="ep2", bufs=2) as ep2,
            tc.tile_pool(name="aggp", bufs=4, space="PSUM") as aggp,
            tc.tile_pool(name="zp", bufs=2, space="PSUM") as zp,
            tc.tile_pool(name="dram", bufs=1, space="DRAM") as dram,
        ):
            # ---- resident constants ----
            idx_t = cpool.tile([128, NI12 // 16], i16)
            invd_t = cpool.tile([1, BPC * 128], bf16)
            scl2_t = cpool.tile([128, BPC], f32)
            scl1_t = cpool.tile([128, BPC], f32)
            w_t = [cpool.tile([D, D], bf16, name=f"w{i}") for i in range(3)]
            brow_t = [cpool.tile([1, D], bf16, name=f"brow{i}") for i in range(3)]
            iden_t = cpool.tile([128, 128], fp8)
            lwb_t = cpool.tile([128, D], f32)
            alpha_t = cpool.tile([128, 1], f32)
            logits_t = cpool.tile([128, BPC], f32)
            hbuf_t = cpool.tile([128, BPC, D], bf16, name="h0")
            hbuf = [hbuf_t, hbuf_t]   # in-place: block j overwritten only after its last read
            aggsb = cpool.tile([128, BPC * 128], f32)

            nc.vector.memset(alpha_t[:], NEG_SLOPE)
            for dst, src in [(idx_t, idx_d),
                             (invd_t, invd_d),
                             (scl2_t, scl2_d), (scl1_t, scl1_d),
                             (w_t[0], w_d[0]), (w_t[1], w_d[1]), (w_t[2], w_d[2]),
                             (brow_t[0], brow_d[0]), (brow_t[1], brow_d[1]),
                             (brow_t[2], brow_d[2]), (iden_t, iden_d),
                             (lwb_t, lwb_d)]:
                nc.sync.dma_start(dst[:], src[:])

            bounce = [dram.tile([PC_SLOTS, D], bf16, name=f"bounce{i}") for i in range(2)]
            hfp = [[dram.tile([PIECE_ROWS, D], bf16, name=f"hfp{i}_{p}",
                              addr_space="Shared") for p in range(NSB)]
                   for i in range(2)]
            qsem = [nc.alloc_semaphore(f"qsem{q}") for q in range(NQ)]
            agsem = [nc.alloc_semaphore(f"agsem{i}") for i in range(2)]

            def epilogue(L, j, z, h_in_blk):
                if L < 2:
                    hn = hbuf[L % 2][:, j, :]
                    nc.scalar.activation(hn, z[:], AF.Prelu,
                                         scale=scl2_t[:, j:j + 1],
                                         alpha=alpha_t[:, 0:1])
                    nc.sync.dma_start(
                        bounce[L].rearrange("(b p) d -> b p d", p=128)[j], hn)
                else:
                    h3 = ep2.tile([128, D], f32, tag="h3")
                    nc.scalar.activation(h3[:], z[:], AF.Prelu,
                                         scale=scl1_t[:, j:j + 1],
                                         alpha=alpha_t[:, 0:1])
                    tmp = ep2.tile([128, D], f32, tag="lg")
                    nc.vector.tensor_tensor(tmp[:], h3[:], lwb_t[:], op=ALU.mult)
                    nc.vector.reduce_sum(logits_t[:, j:j + 1], tmp[:],
                                         axis=mybir.AxisListType.X)

            def z_phase(L, j, aggs, h_in_blk):
                z = zp.tile([128, 128], f32, tag="z")
                nc.tensor.matmul(z[:], aggs, w_t[L][:], start=True, stop=False)
                nc.tensor.matmul(z[:], invd_t[:, j * 128:(j + 1) * 128],
                                 brow_t[L][:], start=False, stop=(L == 0))
                if L > 0:
                    diagb = ep.tile([128, 128], bf16, tag="diagb")
                    nc.sync.dma_start(diagb[:], diag_d[:, j * 128:(j + 1) * 128])
                    nc.tensor.matmul(z[:], diagb[:],
                                     h_in_blk, start=False, stop=True)
                epilogue(L, j, z, h_in_blk)

            GMAX0B = int(cpb0.max())

            # ================= LAYER 0 (block-major) =================
            for j in range(BPC):
                ng = int(cpb0[j])
                gbase = int(off0[j])
                xg = g0pool.tile([128, GMAX0B, D], bf16, tag="g0", name="xg")
                nc.sync.dma_start(
                    xg[:, :ng, :],
                    xg0_d[:, gbase * D:(gbase + ng) * D]
                    .rearrange("p (g d) -> p g d", d=D))
                oh = oh0p.tile([128, GMAX0B * 128], fp8, tag="oh0", name="oh0")
                nc.sync.dma_start(oh[:, :ng * 128],
                                  oh0_d[:, gbase * 128:(gbase + ng) * 128])
                xb = ep.tile([128, D], bf16, tag="xb")
                nc.sync.dma_start(xb[:], xsl_d[:, j * D:(j + 1) * D])
                agg = aggp.tile([128, 128], f32, tag="agg")
                for t in range(ng):
                    nc.tensor.matmul(agg[:], xg[:, t, :],
                                     oh[:, t * 128:(t + 1) * 128],
                                     start=(t == 0), stop=False)
                nc.tensor.matmul(agg[:], xb[:], iden_t[:],
                                 start=(ng == 0), stop=True)
                aggs = ep.tile([128, D], bf16, tag="aggs")
                nc.scalar.activation(aggs[:], agg[:], AF.Copy)
                z_phase(0, j, aggs[:], xb[:])

            # cumulative fired-gather count per queue, for explicit
            # gather-completion waits on the consumer side
            fired = [0] * NQ
            fired_at = {}   # (L, s) -> [per-queue cumulative count after s]

            # ============ LAYERS 1-2 (source-piece passes) ============
            for L in (1, 2)[:max(0, _nlayers - 1)]:
                h_in = hbuf[(L - 1) % 2]

                # -- gpsimd stream: AG piece s of the PREVIOUS layer's
                #    output interleaves with this layer's preps/triggers so
                #    descgen streams continuously. --
                g_sp = []
                for s in range(NSB):
                    gt = gpool.tile([128, GTILE, D], bf16, tag="g", name=f"g{L}_{s}")
                    g_sp.append(gt)
                    nib = int(spbase[s]) * 8   # idx column base (128/16 per grp)
                    # AG piece first; fused gathers follow and wait on its
                    # completion via the standard (v2-proven) resolution.
                    nc.gpsimd.collective_compute(
                        "AllGather", ALU.bypass,
                        replica_groups=[list(range(NCORES))],
                        ins=[bounce[L - 1][s * SB * 128:(s + 1) * SB * 128, :].opt()],
                        outs=[hfp[L - 1][s].opt()])
                    for rnd in call_plan[s]:
                        for (q, ga, gb) in rnd:
                            nidx = (gb - ga) * 128
                            nc.gpsimd.dma_gather(
                                gt[:, ga:gb, :], hfp[L - 1][s][:, :],
                                idx_t[:, nib + ga * 8:nib + gb * 8],
                                num_idxs=nidx, num_idxs_reg=nidx, elem_size=D,
                                single_packet=False, queue_num=q)

                # -- compute passes --
                for s in range(NSB):
                    gt = g_sp[s]
                    oh = ohp.tile([128, GTILE * 128], fp8, tag="oh", name="oh")
                    nc.sync.dma_start(oh[:, :int(G12[s]) * 128],
                                      oh12_d[:, int(spbase[s]) * 128:
                                             (int(spbase[s]) + int(G12[s])) * 128])
                    for j in range(BPC):
                        ngrp = int(cpb12[j, s])
                        if ngrp > 0 or s == 0:
                            o0 = int(off12[j, s])
                            agg = aggp.tile([128, 128], f32, tag="agg")
                            for t in range(ngrp):
                                nc.tensor.matmul(
                                    agg[:], gt[:, o0 + t, :],
                                    oh[:, (o0 + t) * 128:(o0 + t + 1) * 128],
                                    start=(t == 0),
                                    stop=(False if s == 0 else t == ngrp - 1))
                            if s == 0:
                                nc.tensor.matmul(agg[:], h_in[:, j, :], iden_t[:],
                                                 start=(ngrp == 0), stop=True)
                            ac = aggsb[:, j * 128:(j + 1) * 128]
                            if s == 0:
                                nc.vector.tensor_copy(ac, agg[:])
                            else:
                                nc.vector.tensor_tensor(ac, ac, agg[:], op=ALU.add)
                        if s == NSB - 1:
                            # finalize block j
                            ac = aggsb[:, j * 128:(j + 1) * 128]
                            aggs = ep.tile([128, D], bf16, tag="aggs")
                            nc.scalar.activation(aggs[:], ac, AF.Copy)
                            z_phase(L, j, aggs[:], h_in[:, j, :])

            if _nlayers == 3:
                nc.sync.dma_start(out_d.rearrange("(b p) -> p b", p=128), logits_t[:])
            else:
                # debug: dump first feature of last computed prescaled h
                logits_dbg = cpool.tile([128, BPC], f32)
                nc.vector.tensor_copy(logits_dbg[:],
                                      hbuf[(_nlayers - 1) % 2][:, :, 0])
                nc.sync.dma_start(out_d.rearrange("(b p) -> p b", p=128),
                                  logits_dbg[:])

    nc.compile()
    return nc


def kernel(x, edge_index, W1, b1, W2, b2, W3, b3, lw, lb):
    global LAST_EXEC_NS, LAST_RESULTS
    import concourse.bass_utils as bass_utils
    import ml_dtypes

    bf16 = ml_dtypes.bfloat16
    x = np.asarray(x, np.float32)
    pk = _pack_graph(np.asarray(edge_index), x)
    key = (tuple(pk["cpb0"].tolist()), tuple(map(tuple, pk["cpb12"].tolist())))
    if key not in _CACHE:
        _CACHE[key] = _build_program(pk)
    nc = _CACHE[key]

    ws = [np.ascontiguousarray(np.asarray(w, np.float32)).astype(bf16)
          for w in (W1, W2, W3)]
    brows = [np.asarray(b, np.float32).reshape(1, D).astype(bf16)
             for b in (b1, b2, b3)]
    iden = np.eye(128, dtype=np.float32).astype(ml_dtypes.float8_e4m3fn)
    lwb = np.tile(np.asarray(lw, np.float32).reshape(1, D), (128, 1))

    in_maps = []
    for c in range(NCORES):
        in_maps.append({
            "idx12": pk["idx12"][c], "oh12": pk["oh12"][c], "oh0": pk["oh0"][c],
            "xg0": pk["xg0"][c], "xsl": pk["xsl"][c],
            "diag": pk["diag"][c], "invd": pk["invd"][c],
            "scl2": pk["scl2"][c], "scl1": pk["scl1"][c],
            "w1": ws[0], "w2": ws[1], "w3": ws[2],
            "brow1": brows[0], "brow2": brows[1], "brow3": brows[2],
            "iden": iden, "lwb": lwb,
        })

    res = bass_utils.run_bass_kernel_spmd(nc, in_maps, core_ids=list(range(NCORES)))
    LAST_EXEC_NS = res.exec_time_ns
    LAST_RESULTS = res
    out_slots = np.concatenate([res.results[c]["out"] for c in range(NCORES)])
    logits = out_slots[pk["slot_of"]].astype(np.float32)
    return logits + np.float32(np.asarray(lb).reshape(-1)[0])


# revision 31
# speedup vs baseline: 1.0021x; 1.0021x over previous
"""3-layer GCN (GCNConv x3 + linear head) on 8 Trainium2 NeuronCores.

v3 strategy (continuous-descgen pull design):
  - Nodes bin-packed into 392 blocks of <=128 (balanced by in-edge count),
    49 blocks/core, grouped into 7 superblocks (=source "pieces") of 7
    blocks. Features bf16, PRESCALED hfull[n] = dis[n]*H[n]; leaky-relu
    positive homogeneity folds dis factors into one Prelu epilogue per
    block (as v2).
  - Halo exchange is PIECEWISE: 7 AllGathers per layer boundary, one per
    superblock, each into its own Shared DRAM tensor hfp[L][p]
    ([8*896, 128] piece-major), issued as soon as that superblock's
    outputs land. Gather indices are piece-relative (< 7168, int16).
  - Layers 1-2 gather source rows with SWDGE dma_gather in PREPARE_ONLY
    mode: descriptor generation (the HW bottleneck, ~2.2-3 ns/desc
    aggregate over 4 queues) streams continuously on the GpSimd engine
    from t~0, while trigger_dma fires each source-piece's descriptors
    once its AllGather piece lands. Ring: dynamic_dma_scratch 32KiB ->
    2048 desc/queue; calls sized ~13 groups (1664 descs) so one round
    fits the ring untriggered.
  - Compute for layers 1-2 runs in 7 source-piece passes: per (block,
    piece) chunk matmuls accumulate in PSUM, then DVE adds into a
    per-block f32 SBUF accumulator aggT [d, c]. Pass 0 adds the
    self-loop (identity rhs); the last pass interleaves the finalize
    (z = aggW + bias-outer + diag*h_in, Prelu) so outputs stream out
    block-by-block and the next AllGather pieces pipeline.
  - One-hot chunk matrices are NOT stored: the DVE generates them per
    piece with a broadcast is_equal against a per-chunk target-column
    table (iota row vs colloc), output fp8. Saves ~12 MB/layer of HBM
    streaming and ~80 KB/partition of SBUF.
  - Layer 0 needs no gather: host pregathers dis[src]*x[src] in chunk
    order; block-major compute over superblocks, bounce writes feed the
    first piecewise AllGather.
"""

import numpy as np

N = 50000
E = 600000
D = 128
NCORES = 8
BPC = 49                      # blocks per core
NBLK = NCORES * BPC           # 392
PC_SLOTS = BPC * 128          # 6272
SB = 7                        # blocks per superblock
NSB = BPC // SB               # 7 superblocks (= source pieces)
PIECE_ROWS = NCORES * SB * 128  # 7168 rows per hfp piece
NEG_SLOPE = 0.2
NQ = 4                        # SWDGE queues
SCRATCH = 57344               # dma scratch bytes/partition -> 3584 desc/queue

_CACHE = {}
LAST_EXEC_NS = None
LAST_RESULTS = None


def _pack_graph(edge_index, x):
    import heapq
    import ml_dtypes

    bf16 = ml_dtypes.bfloat16

    row = np.ascontiguousarray(edge_index[0]).astype(np.int64)
    col = np.ascontiguousarray(edge_index[1]).astype(np.int64)
    deg_t = np.bincount(col, minlength=N).astype(np.int64)
    dis = (1.0 / np.sqrt(deg_t + 1.0)).astype(np.float64)

    # --- node -> (block, pos): greedy balanced bin packing by in-degree ---
    order = np.argsort(-deg_t, kind="stable")
    heap = [(0, b) for b in range(NBLK)]
    heapq.heapify(heap)
    nodecnt = np.zeros(NBLK, np.int64)
    load = np.zeros(NBLK, np.int64)
    blk_of = np.empty(N, np.int64)
    pos_of = np.empty(N, np.int64)
    for n in order:
        while True:
            _, b = heapq.heappop(heap)
            if nodecnt[b] < 128:
                break
        blk_of[n] = b
        pos_of[n] = nodecnt[b]
        nodecnt[b] += 1
        load[b] += deg_t[n]
        heapq.heappush(heap, (load[b], b))

    c_of = blk_of // BPC
    bl49_of = blk_of % BPC
    sbn_of = bl49_of // SB
    bl7_of = bl49_of % SB
    # gather offset within the node's piece tensor hfp[sbn]
    poff_of = c_of * (SB * 128) + bl7_of * 128 + pos_of
    # v2-style output slot (block-major per core), for unshard
    slot_of = blk_of * 128 + pos_of

    # --- edge grouping ---
    tb = blk_of[col]                      # target block 0..391
    sp = sbn_of[row]                      # source piece 0..6
    key = tb * NSB + sp
    eorder = np.argsort(key, kind="stable")
    key_s = key[eorder]
    kstart = np.searchsorted(key_s, np.arange(NBLK * NSB + 1))

    e_cnt = (kstart[1:] - kstart[:-1]).reshape(NBLK, NSB)   # [tb, sp]
    tot_cnt = e_cnt.sum(axis=1)                              # [tb]

    # chunks per (bl49, sp): max over cores so the program is SPMD-shared
    cpb12 = np.zeros((BPC, NSB), np.int64)
    for bl49 in range(BPC):
        for s in range(NSB):
            cnts = e_cnt[bl49::BPC, s] if False else e_cnt[np.arange(NCORES) * BPC + bl49, s]
            cpb12[bl49, s] = int(np.ceil(cnts.max() / 128)) if cnts.max() else 0
    # L0 chunks per bl49 (max over cores of total)
    cpb0 = np.zeros(BPC, np.int64)
    for bl49 in range(BPC):
        cpb0[bl49] = int(np.ceil(tot_cnt[np.arange(NCORES) * BPC + bl49].max() / 128))

    G12 = cpb12.sum(axis=0)               # groups per sp piece
    G12MAX = int(G12.max())
    NI12 = int(G12.sum()) * 128           # gather idxs per core (padded)
    G0 = int(cpb0.sum())                  # L0 groups total
    G0_sb = [int(cpb0[s * SB:(s + 1) * SB].sum()) for s in range(NSB)]
    G0SBMAX = max(G0_sb)

    # group offsets
    off12 = np.zeros((BPC, NSB), np.int64)   # group offset within piece sp
    for s in range(NSB):
        off12[:, s] = np.cumsum(cpb12[:, s]) - cpb12[:, s]
    spbase = np.cumsum(G12) - G12            # piece group base (global chunks)
    off0 = np.cumsum(cpb0) - cpb0            # L0 group offset

    # --- per-core tensors ---
    import ml_dtypes as _mld
    fp8 = _mld.float8_e4m3fn
    TG12 = int(G12.sum())
    idx12 = np.zeros((NCORES, NI12), np.int16)
    oh12 = np.zeros((NCORES, 128, TG12 * 128), fp8)
    oh0 = np.zeros((NCORES, 128, G0 * 128), fp8)
    xs = np.asarray(x, np.float64) * dis[:, None]
    xs_bf = xs.astype(bf16)
    xg0 = np.zeros((NCORES, 128, G0, D), bf16)

    for b in range(NBLK):
        cc = b // BPC
        bl49 = b % BPC
        # layer 1-2: per source piece
        for s in range(NSB):
            sub = eorder[kstart[b * NSB + s]:kstart[b * NSB + s + 1]]
            if len(sub) == 0:
                continue
            i = np.arange(len(sub))
            t = i // 128
            p = i % 128
            g = off12[bl49, s] + t
            idx12[cc, (int(spbase[s]) + g) * 128 + p] = poff_of[row[sub]].astype(np.int16)
            oh12[cc][p, (int(spbase[s]) + g) * 128 + pos_of[col[sub]]] = np.float32(1.0)
        # layer 0: all edges of block, pregathered stream
        sub = eorder[kstart[b * NSB]:kstart[(b + 1) * NSB]]
        if len(sub):
            i = np.arange(len(sub))
            t = i // 128
            p = i % 128
            g0 = off0[bl49] + t
            xg0[cc, p, g0, :] = xs_bf[row[sub]]
            oh0[cc][p, g0 * 128 + pos_of[col[sub]]] = np.float32(1.0)

    # wrap idx: [NCORES, NI] -> [NCORES, 128, NI/16]
    def wrap(a):
        ncc, ni = a.shape
        w = a.reshape(ncc, ni // 16, 16).transpose(0, 2, 1)        # [c,16,ni/16]
        w = np.tile(w, (1, 8, 1))                                   # [c,128,ni/16]
        return np.ascontiguousarray(w)

    # per-block column tensors (v2 verbatim)
    dis_slots = np.ones(NBLK * 128, np.float64)
    dis_slots[slot_of] = dis
    invdis2_slots = np.ones(NBLK * 128, np.float64)
    invdis2_slots[slot_of] = deg_t + 1.0
    xs_slots = np.zeros((NBLK * 128, D), np.float64)
    xs_slots[slot_of] = xs
    xsl = xs_slots.astype(bf16).reshape(NCORES, BPC, 128, D).transpose(0, 2, 1, 3)
    xsl = np.ascontiguousarray(xsl.reshape(NCORES, 128, BPC * D))

    dis_b = dis_slots.reshape(NCORES, BPC, 128).transpose(0, 2, 1)
    scl2 = np.ascontiguousarray((dis_b * dis_b).astype(np.float32))
    scl1 = np.ascontiguousarray(dis_b.astype(np.float32))
    diag = np.zeros((NCORES, 128, BPC * 128), bf16)
    invd = np.zeros((NCORES, 1, BPC * 128), bf16)
    iv2 = invdis2_slots.reshape(NCORES, BPC, 128)
    for cc in range(NCORES):
        for j in range(BPC):
            dg = iv2[cc, j]
            diag[cc, np.arange(128), j * 128 + np.arange(128)] = dg.astype(bf16)
            invd[cc, 0, j * 128:(j + 1) * 128] = np.sqrt(dg).astype(bf16)

    return dict(
        slot_of=slot_of,
        cpb12=cpb12, cpb0=cpb0, G12=G12, G12MAX=G12MAX, NI12=NI12,
        G0=G0, G0_sb=G0_sb, G0SBMAX=G0SBMAX, off12=off12, off0=off0,
        spbase=spbase, TG12=TG12,
        idx12=wrap(idx12), oh12=oh12, oh0=oh0,
        xg0=xg0.reshape(NCORES, 128, G0 * D),
        xsl=xsl, scl2=scl2, scl1=scl1, diag=diag, invd=invd,
    )


def _build_program(pk):
    import concourse.bacc as bacc
    import concourse.tile as tile
    import concourse.mybir as mybir

    f32 = mybir.dt.float32
    bf16 = mybir.dt.bfloat16
    fp8 = mybir.dt.float8e4
    i16 = mybir.dt.int16
    ALU = mybir.AluOpType
    AF = mybir.ActivationFunctionType

    cpb12 = pk["cpb12"]; cpb0 = pk["cpb0"]
    G12 = pk["G12"]; G12MAX = pk["G12MAX"]; NI12 = pk["NI12"]
    G0 = pk["G0"]; G0_sb = pk["G0_sb"]; G0SBMAX = pk["G0SBMAX"]
    off12 = pk["off12"]; off0 = pk["off0"]; spbase = pk["spbase"]
    TG12 = pk["TG12"]

    GTILE = G12MAX

    import os as _os
    _nlayers = int(_os.environ.get("GNN3_LAYERS", "3"))

    nc = bacc.Bacc("TRN2", target_bir_lowering=False, debug=False,
                   enable_asserts=True, num_devices=NCORES,
                   num_swdge_queues=NQ, dynamic_dma_scratch_size=SCRATCH)

    idx_d = nc.dram_tensor("idx12", [128, NI12 // 16], i16, kind="ExternalInput").ap()
    oh12_d = nc.dram_tensor("oh12", [128, TG12 * 128], fp8, kind="ExternalInput").ap()
    oh0_d = nc.dram_tensor("oh0", [128, G0 * 128], fp8, kind="ExternalInput").ap()
    xg0_d = nc.dram_tensor("xg0", [128, G0 * D], bf16, kind="ExternalInput").ap()
    xsl_d = nc.dram_tensor("xsl", [128, BPC * D], bf16, kind="ExternalInput").ap()
    diag_d = nc.dram_tensor("diag", [128, BPC * 128], bf16, kind="ExternalInput").ap()
    invd_d = nc.dram_tensor("invd", [1, BPC * 128], bf16, kind="ExternalInput").ap()
    scl2_d = nc.dram_tensor("scl2", [128, BPC], f32, kind="ExternalInput").ap()
    scl1_d = nc.dram_tensor("scl1", [128, BPC], f32, kind="ExternalInput").ap()
    w_d = [nc.dram_tensor(f"w{i}", [D, D], bf16, kind="ExternalInput").ap() for i in (1, 2, 3)]
    brow_d = [nc.dram_tensor(f"brow{i}", [1, D], bf16, kind="ExternalInput").ap() for i in (1, 2, 3)]
    iden_d = nc.dram_tensor("iden", [128, 128], fp8, kind="ExternalInput").ap()
    lwb_d = nc.dram_tensor("lwb", [128, D], f32, kind="ExternalInput").ap()
    out_d = nc.dram_tensor("out", [PC_SLOTS], f32, kind="ExternalOutput").ap()

    # per (sp): two rounds of one sub-gather call per queue (~1664
    # descs/call; one round fits the ring untriggered).
    NCALLS = 2 * NQ
    call_plan = []  # [sp] -> list of rounds of (q, ga, gb)
    for s in range(NSB):
        g = int(G12[s])
        spans = []
        base = 0
        for ci in range(NCALLS):
            take = (g - base + (NCALLS - ci) - 1) // (NCALLS - ci)
            spans.append((base, base + take))
            base += take
        rounds = []
        for r in range(2):
            rounds.append([(q, spans[r * NQ + q][0], spans[r * NQ + q][1])
                           for q in range(NQ)
                           if spans[r * NQ + q][1] > spans[r * NQ + q][0]])
        call_plan.append(rounds)

    with tile.TileContext(nc) as tc:
        with (
            tc.tile_pool(name="const", bufs=1) as cpool,
            tc.tile_pool(name="gpool", bufs=2) as gpool,      # L1/L2 gather tiles
            tc.tile_pool(name="g0pool", bufs=2) as g0pool,    # L0 per-block streams
            tc.tile_pool(name="ohp", bufs=2) as ohp,          # L1/L2 one-hots
            tc.tile_pool(name="oh0p", bufs=2) as oh0p,        # L0 per-block one-hots
            tc.tile_pool(name="ep", bufs=4) as ep,
            tc.tile_pool(name="ep2", bufs=2) as ep2,
            tc.tile_pool(name="aggp", bufs=4, space="PSUM") as aggp,
            tc.tile_pool(name="zp", bufs=2, space="PSUM") as zp,
            tc.tile_pool(name="dram", bufs=1, space="DRAM") as dram,
        ):
            # ---- resident constants ----
            idx_t = cpool.tile([128, NI12 // 16], i16)
            invd_t = cpool.tile([1, BPC * 128], bf16)
            scl2_t = cpool.tile([128, BPC], f32)
            scl1_t = cpool.tile([128, BPC], f32)
            w_t = [cpool.tile([D, D], bf16, name=f"w{i}") for i in range(3)]
            brow_t = [cpool.tile([1, D], bf16, name=f"brow{i}") for i in range(3)]
            iden_t = cpool.tile([128, 128], fp8)
            lwb_t = cpool.tile([128, D], f32)
            alpha_t = cpool.tile([128, 1], f32)
            logits_t = cpool.tile([128, BPC], f32)
            hbuf_t = cpool.tile([128, BPC, D], bf16, name="h0")
            hbuf = [hbuf_t, hbuf_t]   # in-place: block j overwritten only after its last read
            aggsb = cpool.tile([128, BPC * 128], f32)

            nc.vector.memset(alpha_t[:], NEG_SLOPE)
            for dst, src in [(idx_t, idx_d),
                             (invd_t, invd_d),
                             (scl2_t, scl2_d), (scl1_t, scl1_d),
                             (w_t[0], w_d[0]), (w_t[1], w_d[1]), (w_t[2], w_d[2]),
                             (brow_t[0], brow_d[0]), (brow_t[1], brow_d[1]),
                             (brow_t[2], brow_d[2]), (iden_t, iden_d),
                             (lwb_t, lwb_d)]:
                nc.sync.dma_start(dst[:], src[:])

            bounce = [dram.tile([PC_SLOTS, D], bf16, name=f"bounce{i}") for i in range(2)]
            hfp = [[dram.tile([PIECE_ROWS, D], bf16, name=f"hfp{i}_{p}",
                              addr_space="Shared") for p in range(NSB)]
                   for i in range(2)]
            qsem = [nc.alloc_semaphore(f"qsem{q}") for q in range(NQ)]
            agsem = [nc.alloc_semaphore(f"agsem{i}") for i in range(2)]

            def epilogue(L, j, z, h_in_blk):
                if L < 2:
                    hn = hbuf[L % 2][:, j, :]
                    nc.scalar.activation(hn, z[:], AF.Prelu,
                                         scale=scl2_t[:, j:j + 1],
                                         alpha=alpha_t[:, 0:1])
                    nc.sync.dma_start(
                        bounce[L].rearrange("(b p) d -> b p d", p=128)[j], hn)
                else:
                    h3 = ep2.tile([128, D], f32, tag="h3")
                    nc.scalar.activation(h3[:], z[:], AF.Prelu,
                                         scale=scl1_t[:, j:j + 1],
                                         alpha=alpha_t[:, 0:1])
                    tmp = ep2.tile([128, D], f32, tag="lg")
                    nc.vector.tensor_tensor(tmp[:], h3[:], lwb_t[:], op=ALU.mult)
                    nc.vector.reduce_sum(logits_t[:, j:j + 1], tmp[:],
                                         axis=mybir.AxisListType.X)

            def z_phase(L, j, aggs, h_in_blk):
                z = zp.tile([128, 128], f32, tag="z")
                nc.tensor.matmul(z[:], aggs, w_t[L][:], start=True, stop=False)
                nc.tensor.matmul(z[:], invd_t[:, j * 128:(j + 1) * 128],
                                 brow_t[L][:], start=False, stop=(L == 0))
                if L > 0:
                    diagb = ep.tile([128, 128], bf16, tag="diagb")
                    nc.sync.dma_start(diagb[:], diag_d[:, j * 128:(j + 1) * 128])
                    nc.tensor.matmul(z[:], diagb[:],
                                     h_in_blk, start=False, stop=True)
                epilogue(L, j, z, h_in_blk)

            GMAX0B = int(cpb0.max())

            # ================= LAYER 0 (block-major) =================
            for j in range(BPC):
                ng = int(cpb0[j])
                gbase = int(off0[j])
                xg = g0pool.tile([128, GMAX0B, D], bf16, tag="g0", name="xg")
                nc.sync.dma_start(
                    xg[:, :ng, :],
                    xg0_d[:, gbase * D:(gbase + ng) * D]
                    .rearrange("p (g d) -> p g d", d=D))
                oh = oh0p.tile([128, GMAX0B * 128], fp8, tag="oh0", name="oh0")
                nc.sync.dma_start(oh[:, :ng * 128],
                                  oh0_d[:, gbase * 128:(gbase + ng) * 128])
                xb = ep.tile([128, D], bf16, tag="xb")
                nc.sync.dma_start(xb[:], xsl_d[:, j * D:(j + 1) * D])
                agg = aggp.tile([128, 128], f32, tag="agg")
                for t in range(ng):
                    nc.tensor.matmul(agg[:], xg[:, t, :],
                                     oh[:, t * 128:(t + 1) * 128],
                                     start=(t == 0), stop=False)
                nc.tensor.matmul(agg[:], xb[:], iden_t[:],
                                 start=(ng == 0), stop=True)
                aggs = ep.tile([128, D], bf16, tag="aggs")
                nc.scalar.activation(aggs[:], agg[:], AF.Copy)
                z_phase(0, j, aggs[:], xb[:])

            # cumulative fired-gather count per queue, for explicit
            # gather-completion waits on the consumer side
            fired = [0] * NQ
            fired_at = {}   # (L, s) -> [per-queue cumulative count after s]

            # ============ LAYERS 1-2 (source-piece passes) ============
            for L in (1, 2)[:max(0, _nlayers - 1)]:
                h_in = hbuf[(L - 1) % 2]

                # -- gpsimd stream: dispatch AllGather pieces AHEAD
                #    (lookahead 2) so they are in flight on the CC cores
                #    while earlier pieces' fused-gather descgen runs; the
                #    fused gather for piece s waits on AG(s) completion via
                #    the standard resolution. --
                LA = 2
                g_sp = [gpool.tile([128, GTILE, D], bf16, tag="g", name=f"g{L}_{s}")
                        for s in range(NSB)]

                def emit_ag(s):
                    nc.gpsimd.collective_compute(
                        "AllGather", ALU.bypass,
                        replica_groups=[list(range(NCORES))],
                        ins=[bounce[L - 1][s * SB * 128:(s + 1) * SB * 128, :].opt()],
                        outs=[hfp[L - 1][s].opt()])

                emitted = 0
                for s in range(NSB):
                    while emitted < min(s + 1 + LA, NSB):
                        emit_ag(emitted)
                        emitted += 1
                    gt = g_sp[s]
                    nib = int(spbase[s]) * 8   # idx column base (128/16 per grp)
                    for rnd in call_plan[s]:
                        for (q, ga, gb) in rnd:
                            nidx = (gb - ga) * 128
                            nc.gpsimd.dma_gather(
                                gt[:, ga:gb, :], hfp[L - 1][s][:, :],
                                idx_t[:, nib + ga * 8:nib + gb * 8],
                                num_idxs=nidx, num_idxs_reg=nidx, elem_size=D,
                                single_packet=False, queue_num=q)

                # -- compute passes --
                for s in range(NSB):
                    gt = g_sp[s]
                    oh = ohp.tile([128, GTILE * 128], fp8, tag="oh", name="oh")
                    nc.sync.dma_start(oh[:, :int(G12[s]) * 128],
                                      oh12_d[:, int(spbase[s]) * 128:
                                             (int(spbase[s]) + int(G12[s])) * 128])
                    for j in range(BPC):
                        ngrp = int(cpb12[j, s])
                        if ngrp > 0 or s == 0:
                            o0 = int(off12[j, s])
                            agg = aggp.tile([128, 128], f32, tag="agg")
                            for t in range(ngrp):
                                nc.tensor.matmul(
                                    agg[:], gt[:, o0 + t, :],
                                    oh[:, (o0 + t) * 128:(o0 + t + 1) * 128],
                                    start=(t == 0),
                                    stop=(False if s == 0 else t == ngrp - 1))
                            if s == 0:
                                nc.tensor.matmul(agg[:], h_in[:, j, :], iden_t[:],
                                                 start=(ngrp == 0), stop=True)
                            ac = aggsb[:, j * 128:(j + 1) * 128]
                            if s == 0:
                                nc.vector.tensor_copy(ac, agg[:])
                            else:
                                nc.vector.tensor_tensor(ac, ac, agg[:], op=ALU.add)
                        if s == NSB - 1:
                            # finalize block j
                            ac = aggsb[:, j * 128:(j + 1) * 128]
                            aggs = ep.tile([128, D], bf16, tag="aggs")
                            nc.scalar.activation(aggs[:], ac, AF.Copy)
                            z_phase(L, j, aggs[:], h_in[:, j, :])

            if _nlayers == 3:
                nc.sync.dma_start(out_d.rearrange("(b p) -> p b", p=128), logits_t[:])
            else:
                # debug: dump first feature of last computed prescaled h
                logits_dbg = cpool.tile([128, BPC], f32)
                nc.vector.tensor_copy(logits_dbg[:],
                                      hbuf[(_nlayers - 1) % 2][:, :, 0])
                nc.sync.dma_start(out_d.rearrange("(b p) -> p b", p=128),
                                  logits_dbg[:])

    nc.compile()
    return nc


def kernel(x, edge_index, W1, b1, W2, b2, W3, b3, lw, lb):
    global LAST_EXEC_NS, LAST_RESULTS
    import concourse.bass_utils as bass_utils
    import ml_dtypes

    bf16 = ml_dtypes.bfloat16
    x = np.asarray(x, np.float32)
    pk = _pack_graph(np.asarray(edge_index), x)
    key = (tuple(pk["cpb0"].tolist()), tuple(map(tuple, pk["cpb12"].tolist())))
    if key not in _CACHE:
        _CACHE[key] = _build_program(pk)
    nc = _CACHE[key]

    ws = [np.ascontiguousarray(np.asarray(w, np.float32)).astype(bf16)
          for w in (W1, W2, W3)]
    brows = [np.asarray(b, np.float32).reshape(1, D).astype(bf16)
             for b in (b1, b2, b3)]
    iden = np.eye(128, dtype=np.float32).astype(ml_dtypes.float8_e4m3fn)
    lwb = np.tile(np.asarray(lw, np.float32).reshape(1, D), (128, 1))

    in_maps = []
    for c in range(NCORES):
        in_maps.append({
            "idx12": pk["idx12"][c], "oh12": pk["oh12"][c], "oh0": pk["oh0"][c],
            "xg0": pk["xg0"][c], "xsl": pk["xsl"][c],
            "diag": pk["diag"][c], "invd": pk["invd"][c],
            "scl2": pk["scl2"][c], "scl1": pk["scl1"][c],
            "w1": ws[0], "w2": ws[1], "w3": ws[2],
            "brow1": brows[0], "brow2": brows[1], "brow3": brows[2],
            "iden": iden, "lwb": lwb,
        })

    res = bass_utils.run_bass_kernel_spmd(nc, in_maps, core_ids=list(range(NCORES)))
    LAST_EXEC_NS = res.exec_time_ns
    LAST_RESULTS = res
    out_slots = np.concatenate([res.results[c]["out"] for c in range(NCORES)])
    logits = out_slots[pk["slot_of"]].astype(np.float32)
    return logits + np.float32(np.asarray(lb).reshape(-1)[0])


# revision 32
# speedup vs baseline: 1.0445x; 1.0423x over previous
"""3-layer GCN (GCNConv x3 + linear head) on 8 Trainium2 NeuronCores.

v3 strategy (continuous-descgen pull design):
  - Nodes bin-packed into 392 blocks of <=128 (balanced by in-edge count),
    49 blocks/core, grouped into 7 superblocks (=source "pieces") of 7
    blocks. Features bf16, PRESCALED hfull[n] = dis[n]*H[n]; leaky-relu
    positive homogeneity folds dis factors into one Prelu epilogue per
    block (as v2).
  - Halo exchange is PIECEWISE: 7 AllGathers per layer boundary, one per
    superblock, each into its own Shared DRAM tensor hfp[L][p]
    ([8*896, 128] piece-major), issued as soon as that superblock's
    outputs land. Gather indices are piece-relative (< 7168, int16).
  - Layers 1-2 gather source rows with SWDGE dma_gather in PREPARE_ONLY
    mode: descriptor generation (the HW bottleneck, ~2.2-3 ns/desc
    aggregate over 4 queues) streams continuously on the GpSimd engine
    from t~0, while trigger_dma fires each source-piece's descriptors
    once its AllGather piece lands. Ring: dynamic_dma_scratch 32KiB ->
    2048 desc/queue; calls sized ~13 groups (1664 descs) so one round
    fits the ring untriggered.
  - Compute for layers 1-2 runs in 7 source-piece passes: per (block,
    piece) chunk matmuls accumulate in PSUM, then DVE adds into a
    per-block f32 SBUF accumulator aggT [d, c]. Pass 0 adds the
    self-loop (identity rhs); the last pass interleaves the finalize
    (z = aggW + bias-outer + diag*h_in, Prelu) so outputs stream out
    block-by-block and the next AllGather pieces pipeline.
  - One-hot chunk matrices are NOT stored: the DVE generates them per
    piece with a broadcast is_equal against a per-chunk target-column
    table (iota row vs colloc), output fp8. Saves ~12 MB/layer of HBM
    streaming and ~80 KB/partition of SBUF.
  - Layer 0 needs no gather: host pregathers dis[src]*x[src] in chunk
    order; block-major compute over superblocks, bounce writes feed the
    first piecewise AllGather.
"""

import numpy as np

N = 50000
E = 600000
D = 128
NCORES = 8
BPC = 49                      # blocks per core
NBLK = NCORES * BPC           # 392
PC_SLOTS = BPC * 128          # 6272
SB = 7                        # blocks per superblock
NSB = BPC // SB               # 7 superblocks (= source pieces)
PIECE_ROWS = NCORES * SB * 128  # 7168 rows per hfp piece
NEG_SLOPE = 0.2
NQ = 4                        # SWDGE queues
SCRATCH = 24576               # dma scratch bytes/partition (fused mode streams)

_CACHE = {}
LAST_EXEC_NS = None
LAST_RESULTS = None


def _pack_graph(edge_index, x):
    import heapq
    import ml_dtypes

    bf16 = ml_dtypes.bfloat16

    row = np.ascontiguousarray(edge_index[0]).astype(np.int64)
    col = np.ascontiguousarray(edge_index[1]).astype(np.int64)
    deg_t = np.bincount(col, minlength=N).astype(np.int64)
    dis = (1.0 / np.sqrt(deg_t + 1.0)).astype(np.float64)

    # --- node -> (block, pos): greedy balanced bin packing by in-degree ---
    order = np.argsort(-deg_t, kind="stable")
    heap = [(0, b) for b in range(NBLK)]
    heapq.heapify(heap)
    nodecnt = np.zeros(NBLK, np.int64)
    load = np.zeros(NBLK, np.int64)
    blk_of = np.empty(N, np.int64)
    pos_of = np.empty(N, np.int64)
    for n in order:
        while True:
            _, b = heapq.heappop(heap)
            if nodecnt[b] < 128:
                break
        blk_of[n] = b
        pos_of[n] = nodecnt[b]
        nodecnt[b] += 1
        load[b] += deg_t[n]
        heapq.heappush(heap, (load[b], b))

    c_of = blk_of // BPC
    bl49_of = blk_of % BPC
    sbn_of = bl49_of // SB
    bl7_of = bl49_of % SB
    # gather offset within the node's piece tensor hfp[sbn]
    poff_of = c_of * (SB * 128) + bl7_of * 128 + pos_of
    # v2-style output slot (block-major per core), for unshard
    slot_of = blk_of * 128 + pos_of

    # --- edge grouping ---
    tb = blk_of[col]                      # target block 0..391
    sp = sbn_of[row]                      # source piece 0..6
    key = tb * NSB + sp
    eorder = np.argsort(key, kind="stable")
    key_s = key[eorder]
    kstart = np.searchsorted(key_s, np.arange(NBLK * NSB + 1))

    e_cnt = (kstart[1:] - kstart[:-1]).reshape(NBLK, NSB)   # [tb, sp]
    tot_cnt = e_cnt.sum(axis=1)                              # [tb]

    # chunks per (bl49, sp): max over cores so the program is SPMD-shared
    cpb12 = np.zeros((BPC, NSB), np.int64)
    for bl49 in range(BPC):
        for s in range(NSB):
            cnts = e_cnt[bl49::BPC, s] if False else e_cnt[np.arange(NCORES) * BPC + bl49, s]
            cpb12[bl49, s] = int(np.ceil(cnts.max() / 128)) if cnts.max() else 0
    # L0 chunks per bl49 (max over cores of total)
    cpb0 = np.zeros(BPC, np.int64)
    for bl49 in range(BPC):
        cpb0[bl49] = int(np.ceil(tot_cnt[np.arange(NCORES) * BPC + bl49].max() / 128))

    G12 = cpb12.sum(axis=0)               # groups per sp piece
    G12MAX = int(G12.max())
    NI12 = int(G12.sum()) * 128           # gather idxs per core (padded)
    G0 = int(cpb0.sum())                  # L0 groups total
    G0_sb = [int(cpb0[s * SB:(s + 1) * SB].sum()) for s in range(NSB)]
    G0SBMAX = max(G0_sb)

    # group offsets
    off12 = np.zeros((BPC, NSB), np.int64)   # group offset within piece sp
    for s in range(NSB):
        off12[:, s] = np.cumsum(cpb12[:, s]) - cpb12[:, s]
    spbase = np.cumsum(G12) - G12            # piece group base (global chunks)
    off0 = np.cumsum(cpb0) - cpb0            # L0 group offset

    # --- per-core tensors ---
    import ml_dtypes as _mld
    fp8 = _mld.float8_e4m3fn
    TG12 = int(G12.sum())
    idx12 = np.zeros((NCORES, NI12), np.int16)
    oh12 = np.zeros((NCORES, 128, TG12 * 128), fp8)
    oh0 = np.zeros((NCORES, 128, G0 * 128), fp8)
    xs = np.asarray(x, np.float64) * dis[:, None]
    xs_bf = xs.astype(bf16)
    xg0 = np.zeros((NCORES, 128, G0, D), bf16)

    for b in range(NBLK):
        cc = b // BPC
        bl49 = b % BPC
        # layer 1-2: per source piece
        for s in range(NSB):
            sub = eorder[kstart[b * NSB + s]:kstart[b * NSB + s + 1]]
            if len(sub) == 0:
                continue
            i = np.arange(len(sub))
            t = i // 128
            p = i % 128
            g = off12[bl49, s] + t
            idx12[cc, (int(spbase[s]) + g) * 128 + p] = poff_of[row[sub]].astype(np.int16)
            oh12[cc][p, (int(spbase[s]) + g) * 128 + pos_of[col[sub]]] = np.float32(1.0)
        # layer 0: all edges of block, pregathered stream
        sub = eorder[kstart[b * NSB]:kstart[(b + 1) * NSB]]
        if len(sub):
            i = np.arange(len(sub))
            t = i // 128
            p = i % 128
            g0 = off0[bl49] + t
            xg0[cc, p, g0, :] = xs_bf[row[sub]]
            oh0[cc][p, g0 * 128 + pos_of[col[sub]]] = np.float32(1.0)

    # wrap idx: [NCORES, NI] -> [NCORES, 128, NI/16]
    def wrap(a):
        ncc, ni = a.shape
        w = a.reshape(ncc, ni // 16, 16).transpose(0, 2, 1)        # [c,16,ni/16]
        w = np.tile(w, (1, 8, 1))                                   # [c,128,ni/16]
        return np.ascontiguousarray(w)

    # per-block column tensors (v2 verbatim)
    dis_slots = np.ones(NBLK * 128, np.float64)
    dis_slots[slot_of] = dis
    invdis2_slots = np.ones(NBLK * 128, np.float64)
    invdis2_slots[slot_of] = deg_t + 1.0
    xs_slots = np.zeros((NBLK * 128, D), np.float64)
    xs_slots[slot_of] = xs
    xsl = xs_slots.astype(bf16).reshape(NCORES, BPC, 128, D).transpose(0, 2, 1, 3)
    xsl = np.ascontiguousarray(xsl.reshape(NCORES, 128, BPC * D))

    dis_b = dis_slots.reshape(NCORES, BPC, 128).transpose(0, 2, 1)
    scl2 = np.ascontiguousarray((dis_b * dis_b).astype(np.float32))
    scl1 = np.ascontiguousarray(dis_b.astype(np.float32))
    diag = np.zeros((NCORES, 128, BPC * 128), bf16)
    invd = np.zeros((NCORES, 1, BPC * 128), bf16)
    iv2 = invdis2_slots.reshape(NCORES, BPC, 128)
    for cc in range(NCORES):
        for j in range(BPC):
            dg = iv2[cc, j]
            diag[cc, np.arange(128), j * 128 + np.arange(128)] = dg.astype(bf16)
            invd[cc, 0, j * 128:(j + 1) * 128] = np.sqrt(dg).astype(bf16)

    return dict(
        slot_of=slot_of,
        cpb12=cpb12, cpb0=cpb0, G12=G12, G12MAX=G12MAX, NI12=NI12,
        G0=G0, G0_sb=G0_sb, G0SBMAX=G0SBMAX, off12=off12, off0=off0,
        spbase=spbase, TG12=TG12,
        idx12=wrap(idx12), oh12=oh12, oh0=oh0,
        xg0=xg0.reshape(NCORES, 128, G0 * D),
        xsl=xsl, scl2=scl2, scl1=scl1, diag=diag, invd=invd,
    )


def _build_program(pk):
    import concourse.bacc as bacc
    import concourse.tile as tile
    import concourse.mybir as mybir

    f32 = mybir.dt.float32
    bf16 = mybir.dt.bfloat16
    fp8 = mybir.dt.float8e4
    i16 = mybir.dt.int16
    ALU = mybir.AluOpType
    AF = mybir.ActivationFunctionType

    cpb12 = pk["cpb12"]; cpb0 = pk["cpb0"]
    G12 = pk["G12"]; G12MAX = pk["G12MAX"]; NI12 = pk["NI12"]
    G0 = pk["G0"]; G0_sb = pk["G0_sb"]; G0SBMAX = pk["G0SBMAX"]
    off12 = pk["off12"]; off0 = pk["off0"]; spbase = pk["spbase"]
    TG12 = pk["TG12"]

    GTILE = G12MAX

    import os as _os
    _nlayers = int(_os.environ.get("GNN3_LAYERS", "3"))

    nc = bacc.Bacc("TRN2", target_bir_lowering=False, debug=False,
                   enable_asserts=True, num_devices=NCORES,
                   num_swdge_queues=NQ, dynamic_dma_scratch_size=SCRATCH)

    idx_d = nc.dram_tensor("idx12", [128, NI12 // 16], i16, kind="ExternalInput").ap()
    oh12_d = nc.dram_tensor("oh12", [128, TG12 * 128], fp8, kind="ExternalInput").ap()
    oh0_d = nc.dram_tensor("oh0", [128, G0 * 128], fp8, kind="ExternalInput").ap()
    xg0_d = nc.dram_tensor("xg0", [128, G0 * D], bf16, kind="ExternalInput").ap()
    xsl_d = nc.dram_tensor("xsl", [128, BPC * D], bf16, kind="ExternalInput").ap()
    diag_d = nc.dram_tensor("diag", [128, BPC * 128], bf16, kind="ExternalInput").ap()
    invd_d = nc.dram_tensor("invd", [1, BPC * 128], bf16, kind="ExternalInput").ap()
    scl2_d = nc.dram_tensor("scl2", [128, BPC], f32, kind="ExternalInput").ap()
    scl1_d = nc.dram_tensor("scl1", [128, BPC], f32, kind="ExternalInput").ap()
    w_d = [nc.dram_tensor(f"w{i}", [D, D], bf16, kind="ExternalInput").ap() for i in (1, 2, 3)]
    brow_d = [nc.dram_tensor(f"brow{i}", [1, D], bf16, kind="ExternalInput").ap() for i in (1, 2, 3)]
    iden_d = nc.dram_tensor("iden", [128, 128], fp8, kind="ExternalInput").ap()
    lwb_d = nc.dram_tensor("lwb", [128, D], f32, kind="ExternalInput").ap()
    out_d = nc.dram_tensor("out", [PC_SLOTS], f32, kind="ExternalOutput").ap()

    # per (sp): two rounds of one sub-gather call per queue (~1664
    # descs/call; one round fits the ring untriggered).
    NCALLS = NQ
    call_plan = []  # [sp] -> list of rounds of (q, ga, gb)
    for s in range(NSB):
        g = int(G12[s])
        spans = []
        base = 0
        for ci in range(NCALLS):
            take = (g - base + (NCALLS - ci) - 1) // (NCALLS - ci)
            spans.append((base, base + take))
            base += take
        call_plan.append([[(q, spans[q][0], spans[q][1])
                           for q in range(NQ)
                           if spans[q][1] > spans[q][0]]])

    with tile.TileContext(nc) as tc:
        with (
            tc.tile_pool(name="const", bufs=1) as cpool,
            tc.tile_pool(name="gpool", bufs=2) as gpool,      # L1/L2 gather tiles
            tc.tile_pool(name="g0pool", bufs=5) as g0pool,    # L0 per-block streams
            tc.tile_pool(name="ohp", bufs=2) as ohp,          # L1/L2 one-hots
            tc.tile_pool(name="oh0p", bufs=5) as oh0p,        # L0 per-block one-hots
            tc.tile_pool(name="ep", bufs=4) as ep,
            tc.tile_pool(name="ep2", bufs=2) as ep2,
            tc.tile_pool(name="aggp", bufs=4, space="PSUM") as aggp,
            tc.tile_pool(name="zp", bufs=2, space="PSUM") as zp,
            tc.tile_pool(name="dram", bufs=1, space="DRAM") as dram,
        ):
            # ---- resident constants ----
            idx_t = cpool.tile([128, NI12 // 16], i16)
            invd_t = cpool.tile([1, BPC * 128], bf16)
            scl2_t = cpool.tile([128, BPC], f32)
            scl1_t = cpool.tile([128, BPC], f32)
            w_t = [cpool.tile([D, D], bf16, name=f"w{i}") for i in range(3)]
            brow_t = [cpool.tile([1, D], bf16, name=f"brow{i}") for i in range(3)]
            iden_t = cpool.tile([128, 128], fp8)
            lwb_t = cpool.tile([128, D], f32)
            alpha_t = cpool.tile([128, 1], f32)
            logits_t = cpool.tile([128, BPC], f32)
            hbuf_t = cpool.tile([128, BPC, D], bf16, name="h0")
            hbuf = [hbuf_t, hbuf_t]   # in-place: block j overwritten only after its last read
            aggsb = cpool.tile([128, BPC * 128], f32)

            nc.vector.memset(alpha_t[:], NEG_SLOPE)
            for dst, src in [(idx_t, idx_d),
                             (invd_t, invd_d),
                             (scl2_t, scl2_d), (scl1_t, scl1_d),
                             (w_t[0], w_d[0]), (w_t[1], w_d[1]), (w_t[2], w_d[2]),
                             (brow_t[0], brow_d[0]), (brow_t[1], brow_d[1]),
                             (brow_t[2], brow_d[2]), (iden_t, iden_d),
                             (lwb_t, lwb_d)]:
                nc.sync.dma_start(dst[:], src[:])

            bounce = [dram.tile([PC_SLOTS, D], bf16, name=f"bounce{i}") for i in range(2)]
            hfp = [[dram.tile([PIECE_ROWS, D], bf16, name=f"hfp{i}_{p}",
                              addr_space="Shared") for p in range(NSB)]
                   for i in range(2)]
            qsem = [nc.alloc_semaphore(f"qsem{q}") for q in range(NQ)]
            agsem = [nc.alloc_semaphore(f"agsem{i}") for i in range(2)]

            def epilogue(L, j, z, h_in_blk):
                if L < 2:
                    hn = hbuf[L % 2][:, j, :]
                    nc.scalar.activation(hn, z[:], AF.Prelu,
                                         scale=scl2_t[:, j:j + 1],
                                         alpha=alpha_t[:, 0:1])
                    nc.sync.dma_start(
                        bounce[L].rearrange("(b p) d -> b p d", p=128)[j], hn)
                else:
                    h3 = ep2.tile([128, D], f32, tag="h3")
                    nc.scalar.activation(h3[:], z[:], AF.Prelu,
                                         scale=scl1_t[:, j:j + 1],
                                         alpha=alpha_t[:, 0:1])
                    tmp = ep2.tile([128, D], f32, tag="lg")
                    nc.vector.tensor_tensor(tmp[:], h3[:], lwb_t[:], op=ALU.mult)
                    nc.vector.reduce_sum(logits_t[:, j:j + 1], tmp[:],
                                         axis=mybir.AxisListType.X)

            def z_phase(L, j, aggs, h_in_blk):
                z = zp.tile([128, 128], f32, tag="z")
                nc.tensor.matmul(z[:], aggs, w_t[L][:], start=True, stop=False)
                nc.tensor.matmul(z[:], invd_t[:, j * 128:(j + 1) * 128],
                                 brow_t[L][:], start=False, stop=(L == 0))
                if L > 0:
                    diagb = ep.tile([128, 128], bf16, tag="diagb")
                    nc.sync.dma_start(diagb[:], diag_d[:, j * 128:(j + 1) * 128])
                    nc.tensor.matmul(z[:], diagb[:],
                                     h_in_blk, start=False, stop=True)
                epilogue(L, j, z, h_in_blk)

            GMAX0B = int(cpb0.max())

            # ================= LAYER 0 (block-major) =================
            for j in range(BPC):
                ng = int(cpb0[j])
                gbase = int(off0[j])
                xg = g0pool.tile([128, GMAX0B, D], bf16, tag="g0", name="xg")
                nc.sync.dma_start(
                    xg[:, :ng, :],
                    xg0_d[:, gbase * D:(gbase + ng) * D]
                    .rearrange("p (g d) -> p g d", d=D))
                oh = oh0p.tile([128, GMAX0B * 128], fp8, tag="oh0", name="oh0")
                nc.sync.dma_start(oh[:, :ng * 128],
                                  oh0_d[:, gbase * 128:(gbase + ng) * 128])
                xb = ep.tile([128, D], bf16, tag="xb")
                nc.sync.dma_start(xb[:], xsl_d[:, j * D:(j + 1) * D])
                agg = aggp.tile([128, 128], f32, tag="agg")
                for t in range(ng):
                    nc.tensor.matmul(agg[:], xg[:, t, :],
                                     oh[:, t * 128:(t + 1) * 128],
                                     start=(t == 0), stop=False)
                nc.tensor.matmul(agg[:], xb[:], iden_t[:],
                                 start=(ng == 0), stop=True)
                aggs = ep.tile([128, D], bf16, tag="aggs")
                nc.vector.tensor_copy(aggs[:], agg[:])
                z_phase(0, j, aggs[:], xb[:])

            # cumulative fired-gather count per queue, for explicit
            # gather-completion waits on the consumer side
            fired = [0] * NQ
            fired_at = {}   # (L, s) -> [per-queue cumulative count after s]

            # ============ LAYERS 1-2 (source-piece passes) ============
            for L in (1, 2)[:max(0, _nlayers - 1)]:
                h_in = hbuf[(L - 1) % 2]

                # -- gpsimd stream: dispatch AllGather pieces AHEAD
                #    (lookahead 2) so they are in flight on the CC cores
                #    while earlier pieces' fused-gather descgen runs; the
                #    fused gather for piece s waits on AG(s) completion via
                #    the standard resolution. --
                LA = 2
                g_sp = [gpool.tile([128, GTILE, D], bf16, tag="g", name=f"g{L}_{s}")
                        for s in range(NSB)]

                def emit_ag(s):
                    nc.gpsimd.collective_compute(
                        "AllGather", ALU.bypass,
                        replica_groups=[list(range(NCORES))],
                        ins=[bounce[L - 1][s * SB * 128:(s + 1) * SB * 128, :].opt()],
                        outs=[hfp[L - 1][s].opt()])

                emitted = 0
                for s in range(NSB):
                    while emitted < min(s + 1 + LA, NSB):
                        emit_ag(emitted)
                        emitted += 1
                    gt = g_sp[s]
                    nib = int(spbase[s]) * 8   # idx column base (128/16 per grp)
                    for rnd in call_plan[s]:
                        for (q, ga, gb) in rnd:
                            nidx = (gb - ga) * 128
                            nc.gpsimd.dma_gather(
                                gt[:, ga:gb, :], hfp[L - 1][s][:, :],
                                idx_t[:, nib + ga * 8:nib + gb * 8],
                                num_idxs=nidx, num_idxs_reg=nidx, elem_size=D,
                                single_packet=False, queue_num=q)

                # -- compute passes --
                for s in range(NSB):
                    gt = g_sp[s]
                    oh = ohp.tile([128, GTILE * 128], fp8, tag="oh", name="oh")
                    nc.sync.dma_start(oh[:, :int(G12[s]) * 128],
                                      oh12_d[:, int(spbase[s]) * 128:
                                             (int(spbase[s]) + int(G12[s])) * 128])
                    for j in range(BPC):
                        ngrp = int(cpb12[j, s])
                        if ngrp > 0 or s == 0:
                            o0 = int(off12[j, s])
                            agg = aggp.tile([128, 128], f32, tag="agg")
                            for t in range(ngrp):
                                nc.tensor.matmul(
                                    agg[:], gt[:, o0 + t, :],
                                    oh[:, (o0 + t) * 128:(o0 + t + 1) * 128],
                                    start=(t == 0),
                                    stop=(False if s == 0 else t == ngrp - 1))
                            if s == 0:
                                nc.tensor.matmul(agg[:], h_in[:, j, :], iden_t[:],
                                                 start=(ngrp == 0), stop=True)
                            ac = aggsb[:, j * 128:(j + 1) * 128]
                            if s == 0:
                                nc.vector.tensor_copy(ac, agg[:])
                            else:
                                nc.vector.tensor_tensor(ac, ac, agg[:], op=ALU.add)
                        if s == NSB - 1:
                            # finalize block j
                            ac = aggsb[:, j * 128:(j + 1) * 128]
                            aggs = ep.tile([128, D], bf16, tag="aggs")
                            nc.vector.tensor_copy(aggs[:], ac)
                            z_phase(L, j, aggs[:], h_in[:, j, :])

            if _nlayers == 3:
                nc.sync.dma_start(out_d.rearrange("(b p) -> p b", p=128), logits_t[:])
            else:
                # debug: dump first feature of last computed prescaled h
                logits_dbg = cpool.tile([128, BPC], f32)
                nc.vector.tensor_copy(logits_dbg[:],
                                      hbuf[(_nlayers - 1) % 2][:, :, 0])
                nc.sync.dma_start(out_d.rearrange("(b p) -> p b", p=128),
                                  logits_dbg[:])

    nc.compile()
    return nc


def kernel(x, edge_index, W1, b1, W2, b2, W3, b3, lw, lb):
    global LAST_EXEC_NS, LAST_RESULTS
    import concourse.bass_utils as bass_utils
    import ml_dtypes

    bf16 = ml_dtypes.bfloat16
    x = np.asarray(x, np.float32)
    pk = _pack_graph(np.asarray(edge_index), x)
    key = (tuple(pk["cpb0"].tolist()), tuple(map(tuple, pk["cpb12"].tolist())))
    if key not in _CACHE:
        _CACHE[key] = _build_program(pk)
    nc = _CACHE[key]

    ws = [np.ascontiguousarray(np.asarray(w, np.float32)).astype(bf16)
          for w in (W1, W2, W3)]
    brows = [np.asarray(b, np.float32).reshape(1, D).astype(bf16)
             for b in (b1, b2, b3)]
    iden = np.eye(128, dtype=np.float32).astype(ml_dtypes.float8_e4m3fn)
    lwb = np.tile(np.asarray(lw, np.float32).reshape(1, D), (128, 1))

    in_maps = []
    for c in range(NCORES):
        in_maps.append({
            "idx12": pk["idx12"][c], "oh12": pk["oh12"][c], "oh0": pk["oh0"][c],
            "xg0": pk["xg0"][c], "xsl": pk["xsl"][c],
            "diag": pk["diag"][c], "invd": pk["invd"][c],
            "scl2": pk["scl2"][c], "scl1": pk["scl1"][c],
            "w1": ws[0], "w2": ws[1], "w3": ws[2],
            "brow1": brows[0], "brow2": brows[1], "brow3": brows[2],
            "iden": iden, "lwb": lwb,
        })

    res = bass_utils.run_bass_kernel_spmd(nc, in_maps, core_ids=list(range(NCORES)))
    LAST_EXEC_NS = res.exec_time_ns
    LAST_RESULTS = res
    out_slots = np.concatenate([res.results[c]["out"] for c in range(NCORES)])
    logits = out_slots[pk["slot_of"]].astype(np.float32)
    return logits + np.float32(np.asarray(lb).reshape(-1)[0])


# revision 33
# speedup vs baseline: 1.9954x; 1.9104x over previous
"""3-layer GCN (GCNConv x3 + linear head) on 8 Trainium2 NeuronCores.

v2 strategy (graph/data parallel):
  - Nodes bin-packed into 392 blocks of <=128 (balanced by in-edge count),
    49 blocks/core. Slot layout = [core][block][pos] so a single AllGather
    of per-core shards produces the full node-feature table.
  - Features are bf16 and stored PRESCALED: hfull[n] = dis[n] * H[n].
    Leaky-relu positive homogeneity folds all dis factors into one ACT
    Prelu epilogue per block:
      Hnext~ = Prelu( (aggB@W + outer(1/dis, b) + diag(1/dis^2)@Hprev~)
                      * dis^2[c], alpha=0.2 )
    (last layer uses scale dis[c] to produce unscaled H3 for the head).
  - Aggregation per 128-target block = sum of one-hot matmuls with the
    gathered source rows as the STATIONARY operand (lhsT) and a BINARY
    fp8 one-hot as the streaming rhs -> PSUM holds aggT [d, c]; then
    z = matmul(aggT, W) needs no transposes. Self-loops are one extra
    matmul with rhs = identity.
  - Source rows for layers 1-2 are fetched by dma_gather from the
    AllGathered bf16 table. Descriptor generation (Q7) is the expensive
    part, so gathers are issued as prepare_only on 4 SWDGE queues (4 Q7
    core pairs work in parallel) during the PREVIOUS layer, and fired
    with trigger_dma after the AllGather lands. Layer 0 needs no gather:
    the host pre-gathers dis[src]*x[src] into edge order and the kernel
    streams it contiguously.
"""

import numpy as np

N = 50000
E = 600000
D = 128
NCORES = 8
BPC = 49                      # blocks per core
NBLK = NCORES * BPC           # 392
PC_SLOTS = BPC * 128          # 6272
SLOTS = NBLK * 128            # 50176
HI_BASE = SLOTS - 32768       # 17408
LO_LIM = 32768
SB = 7                        # blocks per super-block
NSB = BPC // SB               # 7 super-blocks per core
NEG_SLOPE = 0.2

_CACHE = {}
LAST_EXEC_NS = None
LAST_RESULTS = None


def _pack_graph(edge_index, x):
    """Pack nodes/edges; build all per-core host tensors."""
    import heapq
    import ml_dtypes

    bf16 = ml_dtypes.bfloat16
    fp8 = ml_dtypes.float8_e4m3fn

    row = np.ascontiguousarray(edge_index[0]).astype(np.int64)
    col = np.ascontiguousarray(edge_index[1]).astype(np.int64)
    deg_t = np.bincount(col, minlength=N).astype(np.int64)
    dis = (1.0 / np.sqrt(deg_t + 1.0)).astype(np.float64)

    # --- node -> (block, pos): greedy balanced bin packing by in-degree ---
    order = np.argsort(-deg_t, kind="stable")
    heap = [(0, b) for b in range(NBLK)]
    heapq.heapify(heap)
    nodecnt = np.zeros(NBLK, np.int64)
    load = np.zeros(NBLK, np.int64)
    blk_of = np.empty(N, np.int64)
    pos_of = np.empty(N, np.int64)
    for n in order:
        while True:
            _, b = heapq.heappop(heap)
            if nodecnt[b] < 128:
                break
        blk_of[n] = b
        pos_of[n] = nodecnt[b]
        nodecnt[b] += 1
        load[b] += deg_t[n]
        heapq.heappush(heap, (load[b], b))
    slot_of = blk_of * 128 + pos_of

    # per-slot values (pad slots get benign defaults)
    dis_slots = np.ones(SLOTS, np.float64)
    dis_slots[slot_of] = dis
    invdis2_slots = np.ones(SLOTS, np.float64)
    invdis2_slots[slot_of] = deg_t + 1.0

    # --- edge classification ---
    tb = blk_of[col]
    srcslot = slot_of[row]

    eorder = np.argsort(tb, kind="stable")
    tb_s = tb[eorder]
    bstart = np.searchsorted(tb_s, np.arange(NBLK + 1))

    lo_need = np.zeros(NBLK, np.int64)
    hi_need = np.zeros(NBLK, np.int64)
    tot = np.zeros(NBLK, np.int64)
    for b in range(NBLK):
        sub = eorder[bstart[b]:bstart[b + 1]]
        s = srcslot[sub]
        lo_need[b] = int((s < HI_BASE).sum())
        hi_need[b] = int((s >= LO_LIM).sum())
        tot[b] = len(sub)
    cpb = int(np.ceil(tot.max() / 128))
    k_lo = int(np.ceil(lo_need.max() / 128)) if lo_need.max() else 0
    k_hi = int(np.ceil(hi_need.max() / 128)) if hi_need.max() else 0
    while k_lo + k_hi < cpb:
        if k_lo <= k_hi:
            k_lo += 1
        else:
            k_hi += 1
    cpb = k_lo + k_hi

    ni_lo = SB * k_lo * 128   # idxs per lo piece (per superblock)
    ni_hi = SB * k_hi * 128
    nchunk = BPC * cpb        # gathered chunks per core per layer

    # flat (pre-wrap) idx arrays and chunk->slot bookkeeping
    idxlo = np.zeros((NCORES, NSB, ni_lo), np.int16)
    idxhi = np.zeros((NCORES, NSB, ni_hi), np.int16)
    oh = np.zeros((NCORES, 128, nchunk * 128), fp8)

    for b in range(NBLK):
        sub = eorder[bstart[b]:bstart[b + 1]]
        s = srcslot[sub]
        m_lo = sub[s < HI_BASE]
        m_hi = sub[s >= LO_LIM]
        m_mid = sub[(s >= HI_BASE) & (s < LO_LIM)]
        lo_n = int(np.clip(len(sub) - 128 * k_hi, len(m_lo), 128 * k_lo))
        take = lo_n - len(m_lo)
        lo_e = np.concatenate([m_lo, m_mid[:take]])
        hi_e = np.concatenate([m_mid[take:], m_hi])
        assert len(lo_e) <= 128 * k_lo and len(hi_e) <= 128 * k_hi

        cc, bl49 = divmod(b, BPC)
        sbn, bl7 = divmod(bl49, SB)
        for half, edges, kk, idxarr, base in (
            (0, lo_e, k_lo, idxlo, 0),
            (1, hi_e, k_hi, idxhi, HI_BASE),
        ):
            ne = len(edges)
            if ne == 0:
                continue
            pos = np.arange(ne)
            t = pos // 128
            p = pos % 128
            ii = (bl7 * kk + t) * 128 + p
            idxarr[cc, sbn, ii] = (srcslot[edges] - base).astype(np.int16)
            cid = bl49 * cpb + (t if half == 0 else k_lo + t)
            colloc = (slot_of[col[edges]] % 128).astype(np.int64)
            oh[cc][p, cid * 128 + colloc] = np.float32(1.0)

    def wrap(a):  # [NCORES, NSB, NI] int16 -> [NCORES, 128, NSB*NI/16]
        ncc, nsb, ni = a.shape
        if ni == 0:
            return np.zeros((ncc, 128, 0), np.int16)
        w = a.reshape(ncc, nsb, ni // 16, 16).transpose(0, 1, 3, 2)
        w = np.tile(w, (1, 1, 8, 1))
        return np.ascontiguousarray(
            w.transpose(0, 2, 1, 3).reshape(ncc, 128, nsb * ni // 16))

    # --- per-core feature-derived arrays ---
    xs = np.asarray(x, np.float64) * dis[:, None]          # H~0 = dis*x
    xs_slots = np.zeros((SLOTS, D), np.float64)
    xs_slots[slot_of] = xs
    xs_bf = xs_slots.astype(bf16)

    # layer-0 pregathered streams, laid out exactly like gather output
    xglo = np.zeros((NCORES, 128, NSB * SB * k_lo * D), bf16)
    xghi = np.zeros((NCORES, 128, NSB * SB * k_hi * D), bf16)
    for cc in range(NCORES):
        for sbn in range(NSB):
            for half, kk, arr, idxarr, base in (
                (0, k_lo, xglo, idxlo, 0),
                (1, k_hi, xghi, idxhi, HI_BASE),
            ):
                if kk == 0:
                    continue
                ids = idxarr[cc, sbn].astype(np.int64) + base   # [SB*kk*128]
                g = xs_bf[ids]                                   # [SB*kk*128, D]
                g = g.reshape(SB * kk, 128, D).transpose(1, 0, 2)
                arr[cc, :, sbn * SB * kk * D:(sbn + 1) * SB * kk * D] = \
                    g.reshape(128, SB * kk * D)

    # own-shard H~0 in [pos, block, d] layout
    xsl = xs_bf.reshape(NCORES, BPC, 128, D).transpose(0, 2, 1, 3)  # [c,128,BPC,D]
    xsl = np.ascontiguousarray(xsl.reshape(NCORES, 128, BPC * D))

    # per-block column tensors
    dis_b = dis_slots.reshape(NCORES, BPC, 128).transpose(0, 2, 1)       # [c,128,BPC]
    scl2 = np.ascontiguousarray((dis_b * dis_b).astype(np.float32))
    scl1 = np.ascontiguousarray(dis_b.astype(np.float32))
    diag = np.zeros((NCORES, 128, BPC * 128), bf16)
    invd = np.zeros((NCORES, 1, BPC * 128), bf16)
    iv2 = invdis2_slots.reshape(NCORES, BPC, 128)
    for cc in range(NCORES):
        for j in range(BPC):
            dg = iv2[cc, j]                          # 1/dis^2 (= deg+1)
            diag[cc, np.arange(128), j * 128 + np.arange(128)] = dg.astype(bf16)
            invd[cc, 0, j * 128:(j + 1) * 128] = np.sqrt(dg).astype(bf16)

    return dict(
        slot_of=slot_of, k_lo=k_lo, k_hi=k_hi, cpb=cpb,
        ni_lo=ni_lo, ni_hi=ni_hi, nchunk=nchunk,
        idxlo=wrap(idxlo), idxhi=wrap(idxhi),
        oh=oh, xglo=xglo, xghi=xghi, xsl=xsl,
        scl2=scl2, scl1=scl1, diag=diag, invd=invd,
    )


def _build_program(k_lo, k_hi, cpb, ni_lo, ni_hi, nchunk):
    import concourse.bacc as bacc
    import concourse.tile as tile
    import concourse.mybir as mybir

    f32 = mybir.dt.float32
    bf16 = mybir.dt.bfloat16
    fp8 = mybir.dt.float8e4
    i16 = mybir.dt.int16
    ALU = mybir.AluOpType
    AF = mybir.ActivationFunctionType

    nc = bacc.Bacc("TRN2", target_bir_lowering=False, debug=False,
                   enable_asserts=True, num_devices=NCORES,
                   num_swdge_queues=4)

    oh_d = nc.dram_tensor("oh", [128, nchunk * 128], fp8, kind="ExternalInput").ap()
    diag_d = nc.dram_tensor("diag", [128, BPC * 128], bf16, kind="ExternalInput").ap()
    invd_d = nc.dram_tensor("invd", [1, BPC * 128], bf16, kind="ExternalInput").ap()
    idxlo_d = nc.dram_tensor("idxlo", [128, NSB * ni_lo // 16], i16, kind="ExternalInput").ap()
    idxhi_d = nc.dram_tensor("idxhi", [128, NSB * ni_hi // 16], i16, kind="ExternalInput").ap()
    xglo_d = nc.dram_tensor("xglo", [128, NSB * SB * k_lo * D], bf16, kind="ExternalInput").ap()
    xghi_d = nc.dram_tensor("xghi", [128, NSB * SB * k_hi * D], bf16, kind="ExternalInput").ap()
    xsl_d = nc.dram_tensor("xsl", [128, BPC * D], bf16, kind="ExternalInput").ap()
    scl2_d = nc.dram_tensor("scl2", [128, BPC], f32, kind="ExternalInput").ap()
    scl1_d = nc.dram_tensor("scl1", [128, BPC], f32, kind="ExternalInput").ap()
    w_d = [nc.dram_tensor(f"w{i}", [D, D], bf16, kind="ExternalInput").ap() for i in (1, 2, 3)]
    brow_d = [nc.dram_tensor(f"brow{i}", [1, D], bf16, kind="ExternalInput").ap() for i in (1, 2, 3)]
    iden_d = nc.dram_tensor("iden", [128, 128], fp8, kind="ExternalInput").ap()
    lwb_d = nc.dram_tensor("lwb", [128, D], f32, kind="ExternalInput").ap()
    out_d = nc.dram_tensor("out", [PC_SLOTS], f32, kind="ExternalOutput").ap()

    import os as _os
    _dbg = _os.environ.get("GNN_DEBUG", "")
    _nlayers = int(_dbg[0]) if _dbg else 3
    _use_coll = "nc" not in _dbg

    with tile.TileContext(nc) as tc:
        with (
            tc.tile_pool(name="const", bufs=1) as cpool,
            tc.tile_pool(name="gpool", bufs=3) as gpool,
            tc.tile_pool(name="ep", bufs=4) as ep,
            tc.tile_pool(name="ep2", bufs=2) as ep2,
            tc.tile_pool(name="aggp", bufs=2, space="PSUM") as aggp,
            tc.tile_pool(name="zp", bufs=2, space="PSUM") as zp,
            tc.tile_pool(name="dram", bufs=1, space="DRAM") as dram,
        ):
            # ---- resident constants ----
            oh_t = cpool.tile([128, nchunk * 128], fp8)
            diag_t = cpool.tile([128, BPC * 128], bf16)
            invd_t = cpool.tile([1, BPC * 128], bf16)
            idxlo_t = cpool.tile([128, NSB * ni_lo // 16], i16)
            idxhi_t = cpool.tile([128, NSB * ni_hi // 16], i16)
            scl2_t = cpool.tile([128, BPC], f32)
            scl1_t = cpool.tile([128, BPC], f32)
            w_t = [cpool.tile([D, D], bf16, name=f"w{i}") for i in range(3)]
            brow_t = [cpool.tile([1, D], bf16, name=f"brow{i}") for i in range(3)]
            iden_t = cpool.tile([128, 128], fp8)
            lwb_t = cpool.tile([128, D], f32)
            alpha_t = cpool.tile([128, 1], f32)
            logits_t = cpool.tile([128, BPC], f32)
            hbuf = [cpool.tile([128, BPC, D], bf16, name=f"h{i}") for i in range(2)]

            nc.vector.memset(alpha_t[:], NEG_SLOPE)
            ohchunk = (nchunk * 128) // NSB
            for _i in range(NSB):
                nc.sync.dma_start(oh_t[:, _i * ohchunk:(_i + 1) * ohchunk],
                                  oh_d[:, _i * ohchunk:(_i + 1) * ohchunk])
            for dst, src in [(idxlo_t, idxlo_d), (idxhi_t, idxhi_d),
                             (diag_t, diag_d), (invd_t, invd_d),
                             (scl2_t, scl2_d), (scl1_t, scl1_d),
                             (w_t[0], w_d[0]), (w_t[1], w_d[1]), (w_t[2], w_d[2]),
                             (brow_t[0], brow_d[0]), (brow_t[1], brow_d[1]),
                             (brow_t[2], brow_d[2]), (iden_t, iden_d),
                             (lwb_t, lwb_d)]:
                nc.sync.dma_start(dst[:], src[:])

            hfull = [dram.tile([SLOTS, D], bf16, name=f"hfull{i}",
                               addr_space="Shared") for i in range(2)]
            bounce = [dram.tile([PC_SLOTS, D], bf16, name=f"bounce{i}") for i in range(2)]

            def piece_q(sbn, half):
                # lo on sbn%4, hi on (sbn+2)%4: each queue gets 2 lo + 2 hi
                # pieces per layer, so per-queue ring occupancy is balanced
                # regardless of the k_lo/k_hi split (fits 1536-desc rings).
                return (sbn + 2 * half) % 4

            # gather tiles for pieces, rotating buffers
            def new_piece_tiles():
                glo = gpool.tile([128, SB * k_lo, D], bf16, tag="glo", name="glo") if k_lo else None
                ghi = gpool.tile([128, SB * k_hi, D], bf16, tag="ghi", name="ghi") if k_hi else None
                return glo, ghi

            def gather_piece(L, sbn):
                # each (sb, half) piece is split into two sub-gathers on
                # different SWDGE queues so all 4 Q7 core pairs generate
                # descriptors concurrently for every superblock.
                src = hfull[L - 1]
                glo, ghi = new_piece_tiles()
                qi = 0
                for kk, gt, idx_t_, ni, base in (
                    (k_lo, glo, idxlo_t, ni_lo, 0),
                    (k_hi, ghi, idxhi_t, ni_hi, HI_BASE),
                ):
                    if not kk:
                        continue
                    srcv = src[base:base + LO_LIM, :]
                    ng = SB * kk               # 128-idx groups in this piece
                    g1 = (ng // 2)             # first sub-piece groups
                    col0 = sbn * (ni // 16)
                    for (ga, gb) in ((0, g1), (g1, ng)):
                        n_sub = (gb - ga) * 128
                        if n_sub == 0:
                            continue
                        nc.gpsimd.dma_gather(
                            gt[:, ga:gb, :], srcv,
                            idx_t_[:, col0 + ga * 8:col0 + gb * 8],
                            num_idxs=n_sub, num_idxs_reg=n_sub, elem_size=D,
                            single_packet=False, queue_num=qi % 4)
                        qi += 1
                return glo, ghi

            def block_compute(L, sbn, bl7, glo, ghi, h_in_blk):
                j = sbn * SB + bl7
                agg = aggp.tile([128, 128], f32, tag="agg")
                for t in range(cpb):
                    if t < k_lo:
                        lhsT = glo[:, bl7 * k_lo + t, :]
                    else:
                        lhsT = ghi[:, bl7 * k_hi + (t - k_lo), :]
                    nc.tensor.matmul(agg[:], lhsT,
                                     oh_t[:, (j * cpb + t) * 128:(j * cpb + t + 1) * 128],
                                     start=(t == 0), stop=False)
                # self loop: aggT += h_in_blk.T
                nc.tensor.matmul(agg[:], h_in_blk, iden_t[:],
                                 start=False, stop=True)
                aggs = ep.tile([128, D], bf16, tag="aggs")
                nc.scalar.activation(aggs[:], agg[:], AF.Copy)
                z = zp.tile([128, 128], f32, tag="z")
                nc.tensor.matmul(z[:], aggs[:], w_t[L][:], start=True, stop=False)
                nc.tensor.matmul(z[:], invd_t[:, j * 128:(j + 1) * 128],
                                 brow_t[L][:], start=False, stop=(L == 0))
                if L > 0:
                    nc.tensor.matmul(z[:], diag_t[:, j * 128:(j + 1) * 128],
                                     h_in_blk, start=False, stop=True)
                if L < _nlayers - 1 or _nlayers < 3:
                    hn = hbuf[L % 2][:, j, :]
                    nc.scalar.activation(hn, z[:], AF.Prelu,
                                         scale=scl2_t[:, j:j + 1],
                                         alpha=alpha_t[:, 0:1])
                    nc.sync.dma_start(
                        bounce[L % 2].rearrange("(b p) d -> b p d", p=128)[j], hn)
                else:
                    h3 = ep2.tile([128, D], f32, tag="h3")
                    nc.scalar.activation(h3[:], z[:], AF.Prelu,
                                         scale=scl1_t[:, j:j + 1],
                                         alpha=alpha_t[:, 0:1])
                    tmp = ep2.tile([128, D], f32, tag="lg")
                    nc.vector.tensor_tensor(tmp[:], h3[:], lwb_t[:], op=ALU.mult)
                    nc.vector.reduce_sum(logits_t[:, j:j + 1], tmp[:],
                                         axis=mybir.AxisListType.X)

            # ================= LAYER 0 =================
            for sbn in range(NSB):
                glo, ghi = new_piece_tiles()
                if k_lo:
                    nc.sync.dma_start(
                        glo[:], xglo_d[:, sbn * SB * k_lo * D:(sbn + 1) * SB * k_lo * D]
                        .rearrange("p (k d) -> p k d", d=D))
                if k_hi:
                    nc.sync.dma_start(
                        ghi[:], xghi_d[:, sbn * SB * k_hi * D:(sbn + 1) * SB * k_hi * D]
                        .rearrange("p (k d) -> p k d", d=D))
                for bl7 in range(SB):
                    j = sbn * SB + bl7
                    xb = ep2.tile([128, D], bf16, tag="xb")
                    nc.sync.dma_start(xb[:], xsl_d[:, j * D:(j + 1) * D])
                    block_compute(0, sbn, bl7, glo, ghi, xb[:])
            if _nlayers > 1:
                if _use_coll:
                    nc.gpsimd.collective_compute(
                        "AllGather", ALU.bypass,
                        replica_groups=[list(range(NCORES))],
                        ins=[bounce[0].opt()], outs=[hfull[0].opt()])

                # ================= LAYER 1 =================
                for sbn in range(NSB):
                    glo, ghi = gather_piece(1, sbn)
                    for bl7 in range(SB):
                        j = sbn * SB + bl7
                        block_compute(1, sbn, bl7, glo, ghi, hbuf[0][:, j, :])
            if _nlayers > 2:
                if _use_coll:
                    nc.gpsimd.collective_compute(
                        "AllGather", ALU.bypass,
                        replica_groups=[list(range(NCORES))],
                        ins=[bounce[1].opt()], outs=[hfull[1].opt()])

                # ================= LAYER 2 =================
                for sbn in range(NSB):
                    glo, ghi = gather_piece(2, sbn)
                    for bl7 in range(SB):
                        j = sbn * SB + bl7
                        block_compute(2, sbn, bl7, glo, ghi, hbuf[1][:, j, :])

            if _nlayers == 3:
                nc.sync.dma_start(out_d.rearrange("(b p) -> p b", p=128), logits_t[:])
            else:
                # debug: dump first feature of last computed h
                logits_dbg = cpool.tile([128, BPC], f32)
                nc.vector.tensor_copy(logits_dbg[:], hbuf[(_nlayers - 1) % 2][:, :, 0])
                nc.sync.dma_start(out_d.rearrange("(b p) -> p b", p=128), logits_dbg[:])

    nc.compile()
    return nc


def kernel(x, edge_index, W1, b1, W2, b2, W3, b3, lw, lb):
    global LAST_EXEC_NS, LAST_RESULTS
    import concourse.bass_utils as bass_utils
    import ml_dtypes

    bf16 = ml_dtypes.bfloat16
    x = np.asarray(x, np.float32)
    pk = _pack_graph(np.asarray(edge_index), x)
    key = (pk["k_lo"], pk["k_hi"], pk["cpb"])
    if key not in _CACHE:
        _CACHE[key] = _build_program(pk["k_lo"], pk["k_hi"], pk["cpb"],
                                     pk["ni_lo"], pk["ni_hi"], pk["nchunk"])
    nc = _CACHE[key]

    ws = [np.ascontiguousarray(np.asarray(w, np.float32)).astype(bf16)
          for w in (W1, W2, W3)]
    brows = [np.asarray(b, np.float32).reshape(1, D).astype(bf16)
             for b in (b1, b2, b3)]
    iden = np.eye(128, dtype=np.float32).astype(ml_dtypes.float8_e4m3fn)
    lwb = np.tile(np.asarray(lw, np.float32).reshape(1, D), (128, 1))

    in_maps = []
    for c in range(NCORES):
        in_maps.append({
            "oh": pk["oh"][c], "diag": pk["diag"][c], "invd": pk["invd"][c],
            "idxlo": pk["idxlo"][c], "idxhi": pk["idxhi"][c],
            "xglo": pk["xglo"][c], "xghi": pk["xghi"][c], "xsl": pk["xsl"][c],
            "scl2": pk["scl2"][c], "scl1": pk["scl1"][c],
            "w1": ws[0], "w2": ws[1], "w3": ws[2],
            "brow1": brows[0], "brow2": brows[1], "brow3": brows[2],
            "iden": iden, "lwb": lwb,
        })

    res = bass_utils.run_bass_kernel_spmd(nc, in_maps, core_ids=list(range(NCORES)))
    LAST_EXEC_NS = res.exec_time_ns
    LAST_RESULTS = res
    out_slots = np.concatenate([res.results[c]["out"] for c in range(NCORES)])
    logits = out_slots[pk["slot_of"]].astype(np.float32)
    return logits + np.float32(np.asarray(lb).reshape(-1)[0])

